# revision 49
# baseline (speedup 1.0000x reference)
"""Trainium2 Bass kernel for: out = 0.5 * sum_g maxpool4(x @ W.T + b).

Shapes: x [4096, 2048] f32, W [4096, 2048] f32, b [4096] f32 -> out [4096] f32.

Sharding over 8 NeuronCores: 2 batch-groups x 4 out-feature-groups.
Core c = (g, j): batch rows g*2048:(g+1)*2048, out features j*1024:(j+1)*1024.
Each core computes partial row-sums of its pooled quarter; host adds the 4
out-feature partials per batch half (pooling groups of 4 are never split
across cores since 1024 % 4 == 0).

Per-core kernel: y tile layout [batch=128 partitions, out_f free].
Inputs are cast to fp8 e4m3 on host and the PE runs MatmulPerfMode.DoubleRow:
each instruction contracts a 256-deep k-pair (two 128-rows packed along a
[., 2, .] free AP dim) at 2 fp8 rows/cycle -- 2x the bf16 rate, the SBUF
moving-read bandwidth limit; 107 ns per matmul measured on HW. Max-pool +
the final row-sum wash the fp8 quantization noise out to ~5e-3 max rel err
(gate is 2e-2).
  lhsT (stationary) = x^T k-pair slice [128 p, 2, 128 b], rhs (moving) =
  W^T k-pair slice [128 p, 2, 256 o]; out free is 256 = half a PSUM bank.
  Each batch tile owns ONE [128, 1024] psum tile (2 banks, 4 chains).
  start_tensor_calc zeroing is bank-granular (ZERO_REGION_SIZE=2048B), so
  only the even chains (o=0,2) open with start=True -- the odd chains ride
  the pending-zero state of their bank's first write (verified on HW), which
  makes their k=0 matmul replace-instead-of-accumulate with start=False.
  PSUM accumulation stays fp32; the pooling stage is 3 fused VectorE ops per
  batch tile: one [128,1024] bias add (reads both PSUM banks, frees them
  atomically -- which also keeps the Tile scheduler in k-major order so 4
  matmuls share each Ldweights), one maxpool4 3D-AP reduce, one row sum.
  The very last batch tile instead folds its second half's bias into the PE
  via a rank-1 DoubleRow matmul (ones^T stationary @ fp8 bias row) so the
  serial tail after the final matmul is just max + row-sum from PSUM.
  The 0.5 output scale rides the weights; W is additionally scaled by 32 to
  keep fp8 out of subnormals (x ~ N(0,1) needs no scale), and the host
  divides the returned row-sums by 32 (max/sum commute with positive scale).

Loop order: k-major over groups of 4 batch-tiles (8 PSUM banks = 4 b x 2)
so each W^T k-pair DMA is consumed by 16 back-to-back matmuls the moment it
lands. wt k-slices alternate between the Sync and Scalar queues (one queue
sustains only ~96 GB/s; group 0 consumes 2 MiB in ~14 us), xt streams on
GpSimd. All SBUF layouts keep the innermost DMA runs contiguous per
partition (strided first-piece DMAs moved 128B packets at ~10 GB/s and
dominated the v2 startup). Everything (~6.5 MiB/core) stays SBUF-resident.
"""

import sys

if "/opt/trn_rl_repo" not in sys.path:
    sys.path.insert(0, "/opt/trn_rl_repo")

import numpy as np
import ml_dtypes

# Problem constants (hardcoded per harness contract).
B, I, O = 4096, 2048, 4096
KS = 4  # maxpool kernel size
SCALE = 0.5
NB_G, NO_G = 2, 4  # batch groups x out-feature groups = 8 cores
BC = B // NB_G  # 2048 batch rows per core
OC = O // NO_G  # 1024 out features per core
P = 128
KP = I // (2 * P)  # 8 contraction k-pair slices (DoubleRow: 256 deep each)
BT = BC // P  # 16 batch tiles per core
GRP = 4  # batch tiles per k-major group
NG = BT // GRP  # 4 groups
NMM = 256  # DoubleRow matmul out free dim (half bank per chain)
NO_MM = OC // NMM  # 4 matmuls across out features per (k-pair, batch tile)
WS = np.float32(32.0)  # fp8 weight scale (host divides output by this)

_NC_CACHE = {}


def _dedup_ldweights(nc):
    """Remove redundant standalone Ldweights from the compiled module.

    bacc splits every Matmult into Ldweights + Matmult(ldweights=False) with
    no dedup, costing ~16 ns/matmul of PE stream time. When consecutive PE
    Ldweights load the identical stationary AP, the array already holds the
    weights, so sync-free duplicates can be dropped. Only duplicates with no
    semaphore waits/updates are removed (a wait-carrying Ldweights guards a
    real dependency).
    """
    removed = 0
    for f in nc.m.functions:
        for blk in f.blocks:
            insts = list(blk.instructions)
            keep = []
            blk_removed = 0
            last_key = None
            for ins in insts:
                tname = type(ins).__name__
                if tname == "InstLdweights":
                    ap = ins.ins[0]
                    key = (
                        ap.memref,
                        ap.offset,
                        str(ap.ap),
                        str(ap.dtype),
                        str(ins.tile_position),
                        str(ins.tile_size),
                        str(ins.perf_mode),
                        str(ins.is_transpose),
                    )
                    if (
                        key == last_key
                        and not ins.has_wait()
                        and not ins.has_update()
                    ):
                        blk_removed += 1
                        continue
                    last_key = key
                keep.append(ins)
            if blk_removed:
                blk.instructions[:] = keep
                removed += blk_removed
    return removed


def build_bass():
    """Build the (SPMD, per-core) Bass program."""
    from concourse import bacc, tile
    import concourse.mybir as mybir

    f32 = mybir.dt.float32
    bf16 = mybir.dt.bfloat16
    f8 = mybir.dt.float8e4
    DR = mybir.MatmulPerfMode.DoubleRow
    NWARM = 12  # PE warm-up matmuls (p-state pre-ramp) during the DMA wait

    # Bacc (not plain Bass): its compile() runs the TRN2 legalization passes
    # (move_matmul_waits_to_ldweights, generate_event_semaphores) without
    # which walrus rejects matmuls carrying >1 semaphore wait.
    nc = bacc.Bacc(
        "TRN2",
        target_bir_lowering=False,
        debug=False,
        num_devices=NB_G * NO_G,
        enable_asserts=False,
        num_swdge_queues=2,
    )
    xt_d = nc.dram_tensor("xt", [KP, NG, P, GRP, 2, P], f8, kind="ExternalInput")
    wt_d = nc.dram_tensor("wt", [KP, P, NO_MM, 2, NMM], f8, kind="ExternalInput")
    biasrep_d = nc.dram_tensor("biasrep", [P, OC], f32, kind="ExternalInput")
    # Tail aux: [:, :, 0:P] is a rank-1 'ones' stationary (1.0 at p=0,i=0,
    # all m), [:, :, P:] holds fp8 bias for out-feature columns [512:1024).
    # The very last batch tile's o>=2 chains append one DoubleRow matmul
    # (ones^T @ bias-row) so the tail pooling can skip the VectorE bias add
    # and reduce_max straight from PSUM.
    tailaux_d = nc.dram_tensor("tailaux", [P, 2, P + 2 * NMM], f8, kind="ExternalInput")
    out_d = nc.dram_tensor("out", [NG, P, GRP], f32, kind="ExternalOutput")

    with tile.TileContext(nc) as tc:
        with (
            tc.tile_pool(name="wt", bufs=KP) as wt_pool,
            # All 32 xt tiles get their own slot: recycling 16 slots to pace
            # the GpSimd stream was tried and cost +16 us in queue stalls.
            tc.tile_pool(name="xt", bufs=KP * NG) as xt_pool,
            tc.tile_pool(name="misc", bufs=1) as misc_pool,
            tc.tile_pool(name="pool4", bufs=4) as pool4_pool,
            tc.tile_pool(name="pooled", bufs=4) as pooled_pool,
            tc.tile_pool(name="psum", bufs=4, space="PSUM") as psum_pool,
        ):
            wt_sb = []
            xt_sb = {}
            # k-pair 0 arrives in fine-grained pieces so the first matmul
            # (which gates the whole PE stream behind the NEFF preamble) can
            # start after ~96 KB instead of ~320 KB. The two pieces gating
            # the first matmul ride two different engine queues so their
            # first-byte latencies overlap; all pieces are contiguous per
            # partition. (Gating on the full xt tile via GpSimd instead was
            # tried: its software-DGE path lands the tile ~1 us later.)
            w = wt_pool.tile([P, NO_MM, 2, NMM], f8, tag="wt", name="w_0")
            xx = xt_pool.tile([P, GRP, 2, P], f8, tag="xt", name="xx_0_0")
            nc.scalar.dma_start(xx[:, 0, :, :], xt_d[0, 0, :, 0, :, :])
            nc.sync.dma_start(w[:, 0, :, :], wt_d[0, :, 0, :, :])
            nc.gpsimd.dma_start(xx[:, 1:, :, :], xt_d[0, 0, :, 1:, :, :])
            nc.sync.dma_start(w[:, 1:, :, :], wt_d[0, :, 1:, :, :])
            wt_sb.append(w)
            xt_sb[(0, 0)] = xx

            # wt k-slices alternate Sync (even k) / Scalar (odd k) as whole
            # 256 KB contiguous DMAs: one queue sustains ~96 GB/s and group 0
            # consumes a slice every ~1.8 us. (Splitting each slice across
            # both queues was tried three ways -- strided SBUF halves,
            # contiguous DRAM halves on both queues, and two half-DMAs on
            # one queue -- and every variant regressed 1-13 us via packet
            # rate or doubled issue overhead.)
            for k in range(1, KP):
                w = wt_pool.tile([P, NO_MM, 2, NMM], f8, tag="wt", name=f"w_{k}")
                eng = nc.sync if k % 2 == 0 else nc.scalar
                eng.dma_start(w[:], wt_d[k])
                wt_sb.append(w)
            # xt on GpSimd: group 0 first, then biasrep (needed by the first
            # pooling at ~16 us), then the rest.
            for k in range(1, KP):
                xx = xt_pool.tile([P, GRP, 2, P], f8, tag="xt", name=f"xx_{k}_0")
                nc.gpsimd.dma_start(xx[:], xt_d[k, 0])
                xt_sb[(k, 0)] = xx
            biasrep = misc_pool.tile([P, OC], f32)
            nc.gpsimd.dma_start(biasrep[:], biasrep_d[:, :])
            tailaux = misc_pool.tile([P, 2, P + 2 * NMM], f8)
            nc.gpsimd.dma_start(tailaux[:], tailaux_d[:])
            outsb = misc_pool.tile([P, BT], f32)
            # PE p-state warm-up: the clock ramps over ~3 us of sustained PE
            # activity, and the first real matmul can't start before the
            # gating DMAs land at ~9.3 us (6.7 us fixed preamble + DMA path
            # latency). Dummy DoubleRow matmuls on a zeroed scratch tile fill
            # that window so the real stream starts at a warm clock.
            scratch = misc_pool.tile([P, 2, NMM], f8)
            nc.vector.memset(scratch[:], 0)
            for g in range(1, NG):
                for k in range(KP):
                    xx = xt_pool.tile(
                        [P, GRP, 2, P], f8, tag="xt", name=f"xx_{k}_{g}"
                    )
                    nc.gpsimd.dma_start(xx[:], xt_d[k, g])
                    xt_sb[(k, g)] = xx

            def emit_pool_half(g, bb, psums, pooled, h, nh):
                """Bias add + maxpool4 for columns [h/nh .. (h+1)/nh) of a
                batch tile. bf16 intermediates: DVE runs 16-bit at 2x, and
                the quantization is washed out by the final row-sum."""
                w2 = OC // nh
                tsum = pool4_pool.tile(
                    [P, w2], bf16, tag="tsum", name=f"tsum_{g}_{bb}_{h}"
                )
                nc.vector.tensor_add(
                    tsum[:],
                    psums[bb][:, h * w2 : (h + 1) * w2],
                    biasrep[:, h * w2 : (h + 1) * w2],
                )
                nc.vector.reduce_max(
                    pooled[:, h * (w2 // KS) : (h + 1) * (w2 // KS)],
                    tsum[:].rearrange("p (q f) -> p q f", f=KS),
                    axis=mybir.AxisListType.X,
                )

            def emit_pooling(g, bb, psums):
                """Fused bias add + maxpool4 + row-sum for one batch tile.
                The full-width add reads both PSUM banks and frees them
                atomically (which also keeps the Tile scheduler k-major)."""
                pooled = pooled_pool.tile(
                    [P, OC // KS], bf16, tag="pooled", name=f"pooled_{g}_{bb}"
                )
                emit_pool_half(g, bb, psums, pooled, 0, 1)
                col = g * GRP + bb
                nc.vector.reduce_sum(
                    outsb[:, col : col + 1], pooled[:], axis=mybir.AxisListType.X
                )

            def emit_mms(g, k, bb, psums, olo=0, ohi=NO_MM, tail_open=False):
                lhsT = xt_sb[(k, g)][:, bb, :, :]
                for o in range(olo, ohi):
                    nc.tensor.matmul(
                        psums[bb][:, o * NMM : (o + 1) * NMM],
                        lhsT,
                        wt_sb[k][:, o, :, :],
                        # Only the even chains zero their (2048B) bank; the
                        # odd chains' first write lands on pending-zero
                        # bytes, which replaces instead of accumulating.
                        start=(k == 0 and o % 2 == 0),
                        stop=(k == KP - 1 and not tail_open),
                        perf_mode=DR,
                        skip_group_check=(o % 2 == 1),
                    )

            for g in range(NG):
                psums = [
                    psum_pool.tile([P, OC], f32, tag="ps", name=f"ps_{g}_{bb}")
                    for bb in range(GRP)
                ]
                if g == 0:
                    for i in range(NWARM):
                        nc.tensor.matmul(
                            psums[0][:, 0:NMM],
                            scratch[:, :, 0:P],
                            scratch[:],
                            start=True,
                            stop=True,
                            perf_mode=DR,
                        )
                if g < NG - 1:
                    # k-major over the group's 4 batch tiles: each wt k-pair
                    # is reused by 16 back-to-back matmuls the moment it
                    # lands, so group 0 streams from HBM without stalling.
                    # Pooling lands at the group tail and overlaps the next
                    # group's matmuls.
                    for k in range(KP):
                        for bb in range(GRP):
                            emit_mms(g, k, bb, psums)
                            if k == KP - 1:
                                emit_pooling(g, bb, psums)
                else:
                    # Last group: batch-tile-major so each tile's pooling
                    # overlaps the next tile's matmuls instead of piling up
                    # after the final matmul. The very last tile runs its two
                    # 512-wide halves back-to-back so the first half's
                    # pooling overlaps the second half's matmuls, shrinking
                    # the serial tail to half a pooling pass.
                    for bb in range(GRP - 1):
                        for k in range(KP):
                            emit_mms(g, k, bb, psums)
                        emit_pooling(g, bb, psums)
                    # Cols 12-14 are final after bb2's pooling: ship them
                    # now (overlapped) so the end-of-program DMA that gates
                    # the finalize barrier carries only col 15.
                    nc.scalar.dma_start(
                        out_d[g, :, 0 : GRP - 1],
                        outsb[:, g * GRP : (g + 1) * GRP - 1],
                    )
                    bb = GRP - 1
                    pooled = pooled_pool.tile(
                        [P, OC // KS], bf16, tag="pooled", name=f"pooled_{g}_{bb}"
                    )
                    # The o>=2 chains accumulate into a 5th psum tile that
                    # recycles bb0's slot (freed by bb0's pooling long
                    # before): the h=0 pooling read and the h=1 matmul
                    # writes then touch DIFFERENT tiles, so the coarse
                    # WAR serialization (~0.8 us stall) disappears and the
                    # first half's pooling fully overlaps the second half.
                    ps_tail = psum_pool.tile([P, OC], f32, tag="ps", name="ps_tail")

                    def tail_mm(k, o, stop):
                        nc.tensor.matmul(
                            ps_tail[:, (o - 2) * NMM : (o - 1) * NMM],
                            xt_sb[(k, g)][:, bb, :, :],
                            wt_sb[k][:, o, :, :],
                            start=(k == 0 and o == 2),
                            stop=stop,
                            perf_mode=DR,
                            skip_group_check=(o == 3),
                        )

                    emit_mms(g, 0, bb, psums, 0, 2)
                    for o in (2, 3):
                        tail_mm(0, o, False)
                    # The rank-1 bias matmuls (ones^T @ bias-row adds
                    # bias[n] to every batch row) ride right behind the
                    # chain openers -- PSUM accumulation is order-free, and
                    # putting them mid-stream instead of last lets the tail
                    # reduce_max start as soon as the k7 matmuls land.
                    for oh in range(2):
                        nc.tensor.matmul(
                            ps_tail[:, oh * NMM : (oh + 1) * NMM],
                            tailaux[:, :, 0:P],
                            tailaux[:, :, P + oh * NMM : P + (oh + 1) * NMM],
                            start=False,
                            stop=False,
                            perf_mode=DR,
                            skip_group_check=True,
                        )
                    for k in range(1, KP):
                        emit_mms(g, k, bb, psums, 0, 2)
                    emit_pool_half(g, bb, psums, pooled, 0, 2)
                    for k in range(1, KP):
                        for o in (2, 3):
                            tail_mm(k, o, k == KP - 1)
                    # Two 256-wide maxes instead of one 512: the o=2 half
                    # can start as soon as its chain closes, hiding under
                    # the o=3 chain's final matmul.
                    q = OC // KS // 4  # pooled columns per 256-wide quarter
                    for oh in range(2):
                        nc.vector.reduce_max(
                            pooled[:, (2 + oh) * q : (3 + oh) * q],
                            ps_tail[:, oh * NMM : (oh + 1) * NMM].rearrange(
                                "p (q f) -> p q f", f=KS
                            ),
                            axis=mybir.AxisListType.X,
                        )
                    nc.vector.reduce_sum(
                        outsb[:, BT - 1 : BT], pooled[:], axis=mybir.AxisListType.X
                    )
                # Per-group output DMA (contiguous 2 KB in DRAM) so only the
                # last group's small piece sits after the final reduce; from
                # ScalarE, whose queue has drained its wt slices by then.
                # (Splitting the final DMA by partition across two queues
                # halves its transfer but lengthens the finalize barrier by
                # ~1 us -- a second queue to complete/drain -- so keep one.)
                # The last group's DMA gates the finalize barrier by ~2.9 us
                # (path latency + packets); GpSimd's swdge path was tried
                # for it and is ~1.7 us slower still. For the final group,
                # cols 12-14 went out above and only col 15 remains here.
                if g < NG - 1:
                    nc.scalar.dma_start(
                        out_d[g, :, :], outsb[:, g * GRP : (g + 1) * GRP]
                    )
                else:
                    nc.scalar.dma_start(
                        out_d[g, :, GRP - 1 : GRP], outsb[:, BT - 1 : BT]
                    )

    nc.compile()
    _dedup_ldweights(nc)
    return nc


def make_in_maps(x, W, b):
    """Host-side shard + preprocess: transpose, fold 0.5 and 32, cast fp8."""
    x = np.asarray(x, dtype=np.float32)
    W = np.asarray(W, dtype=np.float32)
    b = np.asarray(b, dtype=np.float32)
    f8 = ml_dtypes.float8_e4m3

    xt = np.ascontiguousarray(x.T).astype(f8)  # [I, B]
    wt = np.ascontiguousarray(W.T * (np.float32(SCALE) * WS)).astype(f8)  # [I, O]
    bias = (b * (np.float32(SCALE) * WS)).reshape(1, O)

    # Per-batch-half x slabs: [KP, NG, P, GRP, 2, P]; contraction index
    # (kp*2 + i)*128 + p matches the wt slabs below.
    x_slabs = []
    for g in range(NB_G):
        xg = xt[:, g * BC : (g + 1) * BC]  # [I, BC]
        xr = np.ascontiguousarray(
            xg.reshape(KP, 2, P, NG, GRP, P).transpose(0, 3, 2, 4, 1, 5)
        )
        x_slabs.append(xr)
    # Per-out-feature-quarter W slabs [KP, P, NO_MM, 2, NMM] and replicated
    # bias rows.
    w_slabs = []
    b_slabs = []
    ta_slabs = []
    for j in range(NO_G):
        w_slabs.append(
            np.ascontiguousarray(
                wt[:, j * OC : (j + 1) * OC]
                .reshape(KP, 2, P, NO_MM, NMM)
                .transpose(0, 2, 3, 1, 4)
            )
        )
        b_slabs.append(
            np.ascontiguousarray(
                np.broadcast_to(bias[:, j * OC : (j + 1) * OC], (P, OC))
            )
        )
        ta = np.zeros((P, 2, P + 2 * NMM), dtype=np.float32)
        ta[0, 0, 0:P] = 1.0
        ta[0, 0, P:] = bias[0, j * OC + OC // 2 : (j + 1) * OC]
        ta_slabs.append(ta.astype(f8))

    in_maps = []
    for c in range(NB_G * NO_G):
        g, j = divmod(c, NO_G)
        in_maps.append(
            {
                "xt": x_slabs[g],
                "wt": w_slabs[j],
                "biasrep": b_slabs[j],
                "tailaux": ta_slabs[j],
            }
        )
    return in_maps


def combine_outputs(results):
    """Sum the 4 out-feature partials per batch half -> full [B] output."""
    out = np.zeros(B, dtype=np.float32)
    for c, r in enumerate(results):
        g = c // NO_G
        part = np.asarray(r["out"], dtype=np.float32)  # [NG, P, GRP]
        # batch index within the core = (grp*GRP + bb)*P + p
        out[g * BC : (g + 1) * BC] += part.transpose(0, 2, 1).reshape(BC)
    return out / WS  # undo the fp8 weight scale


def kernel(x, W, b):
    from concourse.bass_utils import run_bass_kernel_spmd

    if "nc" not in _NC_CACHE:
        _NC_CACHE["nc"] = build_bass()
    nc = _NC_CACHE["nc"]
    in_maps = make_in_maps(x, W, b)
    res = run_bass_kernel_spmd(nc, in_maps, core_ids=list(range(NB_G * NO_G)))
    return combine_outputs(res.results)


# revision 51
# speedup vs baseline: 1.0232x; 1.0232x over previous
"""Trainium2 Bass kernel for: out = 0.5 * sum_g maxpool4(x @ W.T + b).

Shapes: x [4096, 2048] f32, W [4096, 2048] f32, b [4096] f32 -> out [4096] f32.

Sharding over 8 NeuronCores: 2 batch-groups x 4 out-feature-groups.
Core c = (g, j): batch rows g*2048:(g+1)*2048, out features j*1024:(j+1)*1024.
Each core computes partial row-sums of its pooled quarter; host adds the 4
out-feature partials per batch half (pooling groups of 4 are never split
across cores since 1024 % 4 == 0).

Per-core kernel: y tile layout [batch=128 partitions, out_f free].
Inputs are cast to fp8 e4m3 on host and the PE runs MatmulPerfMode.DoubleRow:
each instruction contracts a 256-deep k-pair (two 128-rows packed along a
[., 2, .] free AP dim) at 2 fp8 rows/cycle -- 2x the bf16 rate, the SBUF
moving-read bandwidth limit; 107 ns per matmul measured on HW. Max-pool +
the final row-sum wash the fp8 quantization noise out to ~5e-3 max rel err
(gate is 2e-2).
  lhsT (stationary) = x^T k-pair slice [128 p, 2, 128 b], rhs (moving) =
  W^T k-pair slice [128 p, 2, 256 o]; out free is 256 = half a PSUM bank.
  Each batch tile owns ONE [128, 1024] psum tile (2 banks, 4 chains).
  start_tensor_calc zeroing is bank-granular (ZERO_REGION_SIZE=2048B), so
  only the even chains (o=0,2) open with start=True -- the odd chains ride
  the pending-zero state of their bank's first write (verified on HW), which
  makes their k=0 matmul replace-instead-of-accumulate with start=False.
  PSUM accumulation stays fp32; the pooling stage is 3 fused VectorE ops per
  batch tile: one [128,1024] bias add (reads both PSUM banks, frees them
  atomically -- which also keeps the Tile scheduler in k-major order so 4
  matmuls share each Ldweights), one maxpool4 3D-AP reduce, one row sum.
  The very last batch tile instead folds its second half's bias into the PE
  via a rank-1 DoubleRow matmul (ones^T stationary @ fp8 bias row) so the
  serial tail after the final matmul is just max + row-sum from PSUM.
  The 0.5 output scale rides the weights; W is additionally scaled by 32 to
  keep fp8 out of subnormals (x ~ N(0,1) needs no scale), and the host
  divides the returned row-sums by 32 (max/sum commute with positive scale).

Loop order: k-major over groups of 4 batch-tiles (8 PSUM banks = 4 b x 2)
so each W^T k-pair DMA is consumed by 16 back-to-back matmuls the moment it
lands. wt k-slices alternate between the Sync and Scalar queues (one queue
sustains only ~96 GB/s; group 0 consumes 2 MiB in ~14 us), xt streams on
GpSimd. All SBUF layouts keep the innermost DMA runs contiguous per
partition (strided first-piece DMAs moved 128B packets at ~10 GB/s and
dominated the v2 startup). Everything (~6.5 MiB/core) stays SBUF-resident.
"""

import sys

if "/opt/trn_rl_repo" not in sys.path:
    sys.path.insert(0, "/opt/trn_rl_repo")

import numpy as np
import ml_dtypes

# Problem constants (hardcoded per harness contract).
B, I, O = 4096, 2048, 4096
KS = 4  # maxpool kernel size
SCALE = 0.5
NB_G, NO_G = 2, 4  # batch groups x out-feature groups = 8 cores
BC = B // NB_G  # 2048 batch rows per core
OC = O // NO_G  # 1024 out features per core
P = 128
KP = I // (2 * P)  # 8 contraction k-pair slices (DoubleRow: 256 deep each)
BT = BC // P  # 16 batch tiles per core
GRP = 4  # batch tiles per k-major group
NG = BT // GRP  # 4 groups
NMM = 256  # DoubleRow matmul out free dim (half bank per chain)
NO_MM = OC // NMM  # 4 matmuls across out features per (k-pair, batch tile)
WS = np.float32(32.0)  # fp8 weight scale (host divides output by this)

_NC_CACHE = {}


def _dedup_ldweights(nc):
    """Remove redundant standalone Ldweights from the compiled module.

    bacc splits every Matmult into Ldweights + Matmult(ldweights=False) with
    no dedup, costing ~16 ns/matmul of PE stream time. When consecutive PE
    Ldweights load the identical stationary AP, the array already holds the
    weights, so sync-free duplicates can be dropped. Only duplicates with no
    semaphore waits/updates are removed (a wait-carrying Ldweights guards a
    real dependency).
    """
    removed = 0
    for f in nc.m.functions:
        for blk in f.blocks:
            insts = list(blk.instructions)
            keep = []
            blk_removed = 0
            last_key = None
            for ins in insts:
                tname = type(ins).__name__
                if tname == "InstLdweights":
                    ap = ins.ins[0]
                    key = (
                        ap.memref,
                        ap.offset,
                        str(ap.ap),
                        str(ap.dtype),
                        str(ins.tile_position),
                        str(ins.tile_size),
                        str(ins.perf_mode),
                        str(ins.is_transpose),
                    )
                    if (
                        key == last_key
                        and not ins.has_wait()
                        and not ins.has_update()
                    ):
                        blk_removed += 1
                        continue
                    last_key = key
                keep.append(ins)
            if blk_removed:
                blk.instructions[:] = keep
                removed += blk_removed
    return removed


def build_bass():
    """Build the (SPMD, per-core) Bass program."""
    from concourse import bacc, tile
    import concourse.mybir as mybir

    f32 = mybir.dt.float32
    bf16 = mybir.dt.bfloat16
    f8 = mybir.dt.float8e4
    DR = mybir.MatmulPerfMode.DoubleRow
    NWARM = 12  # PE warm-up matmuls (p-state pre-ramp) during the DMA wait

    # Bacc (not plain Bass): its compile() runs the TRN2 legalization passes
    # (move_matmul_waits_to_ldweights, generate_event_semaphores) without
    # which walrus rejects matmuls carrying >1 semaphore wait.
    nc = bacc.Bacc(
        "TRN2",
        target_bir_lowering=False,
        debug=False,
        num_devices=NB_G * NO_G,
        enable_asserts=False,
        num_swdge_queues=2,
    )
    xt_d = nc.dram_tensor("xt", [KP, NG, P, GRP, 2, P], f8, kind="ExternalInput")
    wt_d = nc.dram_tensor("wt", [KP, P, NO_MM, 2, NMM], f8, kind="ExternalInput")
    biasrep_d = nc.dram_tensor("biasrep", [P, OC], f32, kind="ExternalInput")
    # Tail aux: [:, :, 0:P] is a rank-1 'ones' stationary (1.0 at p=0,i=0,
    # all m), [:, :, P:] holds fp8 bias for out-feature columns [512:1024).
    # The very last batch tile's o>=2 chains append one DoubleRow matmul
    # (ones^T @ bias-row) so the tail pooling can skip the VectorE bias add
    # and reduce_max straight from PSUM.
    tailaux_d = nc.dram_tensor("tailaux", [P, 2, P + 2 * NMM], f8, kind="ExternalInput")
    out_d = nc.dram_tensor("out", [NG, P, GRP], f32, kind="ExternalOutput")

    with tile.TileContext(nc) as tc:
        with (
            tc.tile_pool(name="wt", bufs=KP) as wt_pool,
            # All 32 xt tiles get their own slot: recycling 16 slots to pace
            # the GpSimd stream was tried and cost +16 us in queue stalls.
            tc.tile_pool(name="xt", bufs=KP * NG) as xt_pool,
            tc.tile_pool(name="misc", bufs=1) as misc_pool,
            tc.tile_pool(name="pool4", bufs=4) as pool4_pool,
            tc.tile_pool(name="pooled", bufs=4) as pooled_pool,
            tc.tile_pool(name="psum", bufs=4, space="PSUM") as psum_pool,
        ):
            wt_sb = []
            xt_sb = {}
            # k-pair 0 arrives in fine-grained pieces so the first matmul
            # (which gates the whole PE stream behind the NEFF preamble) can
            # start after ~96 KB instead of ~320 KB. The two pieces gating
            # the first matmul ride two different engine queues so their
            # first-byte latencies overlap; all pieces are contiguous per
            # partition. (Gating on the full xt tile via GpSimd instead was
            # tried: its software-DGE path lands the tile ~1 us later.)
            w = wt_pool.tile([P, NO_MM, 2, NMM], f8, tag="wt", name="w_0")
            xx = xt_pool.tile([P, GRP, 2, P], f8, tag="xt", name="xx_0_0")
            nc.scalar.dma_start(xx[:, 0, :, :], xt_d[0, 0, :, 0, :, :])
            nc.sync.dma_start(w[:, 0, :, :], wt_d[0, :, 0, :, :])
            nc.gpsimd.dma_start(xx[:, 1:, :, :], xt_d[0, 0, :, 1:, :, :])
            nc.sync.dma_start(w[:, 1:, :, :], wt_d[0, :, 1:, :, :])
            wt_sb.append(w)
            xt_sb[(0, 0)] = xx

            # wt k-slices alternate Sync (even k) / Scalar (odd k) as whole
            # 256 KB contiguous DMAs: one queue sustains ~96 GB/s and group 0
            # consumes a slice every ~1.8 us. (Splitting each slice across
            # both queues was tried three ways -- strided SBUF halves,
            # contiguous DRAM halves on both queues, and two half-DMAs on
            # one queue -- and every variant regressed 1-13 us via packet
            # rate or doubled issue overhead.)
            for k in range(1, KP):
                w = wt_pool.tile([P, NO_MM, 2, NMM], f8, tag="wt", name=f"w_{k}")
                eng = nc.sync if k % 2 == 0 else nc.scalar
                eng.dma_start(w[:], wt_d[k])
                wt_sb.append(w)
            # xt on GpSimd: group 0 first, then biasrep (needed by the first
            # pooling at ~16 us), then the rest.
            for k in range(1, KP):
                xx = xt_pool.tile([P, GRP, 2, P], f8, tag="xt", name=f"xx_{k}_0")
                nc.gpsimd.dma_start(xx[:], xt_d[k, 0])
                xt_sb[(k, 0)] = xx
            biasrep = misc_pool.tile([P, OC], f32)
            nc.gpsimd.dma_start(biasrep[:], biasrep_d[:, :])
            tailaux = misc_pool.tile([P, 2, P + 2 * NMM], f8)
            nc.gpsimd.dma_start(tailaux[:], tailaux_d[:])
            outsb = misc_pool.tile([P, BT], f32)
            # PE p-state warm-up: the clock ramps over ~3 us of sustained PE
            # activity, and the first real matmul can't start before the
            # gating DMAs land at ~9.3 us (6.7 us fixed preamble + DMA path
            # latency). Dummy DoubleRow matmuls on a zeroed scratch tile fill
            # that window so the real stream starts at a warm clock.
            scratch = misc_pool.tile([P, 2, NMM], f8)
            nc.vector.memset(scratch[:], 0)
            for g in range(1, NG):
                for k in range(KP):
                    xx = xt_pool.tile(
                        [P, GRP, 2, P], f8, tag="xt", name=f"xx_{k}_{g}"
                    )
                    nc.gpsimd.dma_start(xx[:], xt_d[k, g])
                    xt_sb[(k, g)] = xx

            def emit_pool_half(g, bb, psums, pooled, h, nh):
                """Bias add + maxpool4 for columns [h/nh .. (h+1)/nh) of a
                batch tile. bf16 intermediates: DVE runs 16-bit at 2x, and
                the quantization is washed out by the final row-sum."""
                w2 = OC // nh
                tsum = pool4_pool.tile(
                    [P, w2], bf16, tag="tsum", name=f"tsum_{g}_{bb}_{h}"
                )
                nc.vector.tensor_add(
                    tsum[:],
                    psums[bb][:, h * w2 : (h + 1) * w2],
                    biasrep[:, h * w2 : (h + 1) * w2],
                )
                nc.vector.reduce_max(
                    pooled[:, h * (w2 // KS) : (h + 1) * (w2 // KS)],
                    tsum[:].rearrange("p (q f) -> p q f", f=KS),
                    axis=mybir.AxisListType.X,
                )

            def emit_pooling(g, bb, psums):
                """Fused bias add + maxpool4 + row-sum for one batch tile.
                The full-width add reads both PSUM banks and frees them
                atomically (which also keeps the Tile scheduler k-major)."""
                pooled = pooled_pool.tile(
                    [P, OC // KS], bf16, tag="pooled", name=f"pooled_{g}_{bb}"
                )
                emit_pool_half(g, bb, psums, pooled, 0, 1)
                col = g * GRP + bb
                nc.vector.reduce_sum(
                    outsb[:, col : col + 1], pooled[:], axis=mybir.AxisListType.X
                )

            def emit_mms(g, k, bb, psums, olo=0, ohi=NO_MM, tail_open=False):
                lhsT = xt_sb[(k, g)][:, bb, :, :]
                for o in range(olo, ohi):
                    nc.tensor.matmul(
                        psums[bb][:, o * NMM : (o + 1) * NMM],
                        lhsT,
                        wt_sb[k][:, o, :, :],
                        # Only the even chains zero their (2048B) bank; the
                        # odd chains' first write lands on pending-zero
                        # bytes, which replaces instead of accumulating.
                        start=(k == 0 and o % 2 == 0),
                        stop=(k == KP - 1 and not tail_open),
                        perf_mode=DR,
                        skip_group_check=(o % 2 == 1),
                    )

            for g in range(NG):
                psums = [
                    psum_pool.tile([P, OC], f32, tag="ps", name=f"ps_{g}_{bb}")
                    for bb in range(GRP)
                ]
                if g == 0:
                    for i in range(NWARM):
                        nc.tensor.matmul(
                            psums[0][:, 0:NMM],
                            scratch[:, :, 0:P],
                            scratch[:],
                            start=True,
                            stop=True,
                            perf_mode=DR,
                        )
                if g < NG - 1:
                    # k-major over the group's 4 batch tiles: each wt k-pair
                    # is reused by 16 back-to-back matmuls the moment it
                    # lands, so group 0 streams from HBM without stalling.
                    # Pooling lands at the group tail and overlaps the next
                    # group's matmuls.
                    for k in range(KP):
                        for bb in range(GRP):
                            emit_mms(g, k, bb, psums)
                            if k == KP - 1:
                                emit_pooling(g, bb, psums)
                else:
                    # Last group: batch-tile-major so each tile's pooling
                    # overlaps the next tile's matmuls instead of piling up
                    # after the final matmul. The very last tile runs its two
                    # 512-wide halves back-to-back so the first half's
                    # pooling overlaps the second half's matmuls, shrinking
                    # the serial tail to half a pooling pass.
                    for bb in range(GRP - 1):
                        for k in range(KP):
                            emit_mms(g, k, bb, psums)
                        emit_pooling(g, bb, psums)
                    bb = GRP - 1
                    pooled = pooled_pool.tile(
                        [P, OC // KS], bf16, tag="pooled", name=f"pooled_{g}_{bb}"
                    )
                    # The o>=2 chains accumulate into a 5th psum tile that
                    # recycles bb0's slot (freed by bb0's pooling long
                    # before): the h=0 pooling read and the h=1 matmul
                    # writes then touch DIFFERENT tiles, so the coarse
                    # WAR serialization (~0.8 us stall) disappears and the
                    # first half's pooling fully overlaps the second half.
                    ps_tail = psum_pool.tile([P, OC], f32, tag="ps", name="ps_tail")

                    def tail_mm(k, o, stop):
                        nc.tensor.matmul(
                            ps_tail[:, (o - 2) * NMM : (o - 1) * NMM],
                            xt_sb[(k, g)][:, bb, :, :],
                            wt_sb[k][:, o, :, :],
                            start=(k == 0 and o == 2),
                            stop=stop,
                            perf_mode=DR,
                            skip_group_check=(o == 3),
                        )

                    emit_mms(g, 0, bb, psums, 0, 2)
                    for o in (2, 3):
                        tail_mm(0, o, False)
                    # The rank-1 bias matmuls (ones^T @ bias-row adds
                    # bias[n] to every batch row) ride right behind the
                    # chain openers -- PSUM accumulation is order-free, and
                    # putting them mid-stream instead of last lets the tail
                    # reduce_max start as soon as the k7 matmuls land.
                    for oh in range(2):
                        nc.tensor.matmul(
                            ps_tail[:, oh * NMM : (oh + 1) * NMM],
                            tailaux[:, :, 0:P],
                            tailaux[:, :, P + oh * NMM : P + (oh + 1) * NMM],
                            start=False,
                            stop=False,
                            perf_mode=DR,
                            skip_group_check=True,
                        )
                    for k in range(1, KP):
                        emit_mms(g, k, bb, psums, 0, 2)
                    emit_pool_half(g, bb, psums, pooled, 0, 2)
                    for k in range(1, KP):
                        for o in (2, 3):
                            tail_mm(k, o, k == KP - 1)
                    # Two 256-wide maxes instead of one 512: the o=2 half
                    # can start as soon as its chain closes, hiding under
                    # the o=3 chain's final matmul.
                    q = OC // KS // 4  # pooled columns per 256-wide quarter
                    for oh in range(2):
                        nc.vector.reduce_max(
                            pooled[:, (2 + oh) * q : (3 + oh) * q],
                            ps_tail[:, oh * NMM : (oh + 1) * NMM].rearrange(
                                "p (q f) -> p q f", f=KS
                            ),
                            axis=mybir.AxisListType.X,
                        )
                    nc.vector.reduce_sum(
                        outsb[:, BT - 1 : BT], pooled[:], axis=mybir.AxisListType.X
                    )
                # Per-group output DMA (contiguous 2 KB in DRAM) so only the
                # last group's small piece sits after the final reduce; from
                # ScalarE, whose queue has drained its wt slices by then.
                # (Splitting the final DMA by partition across two queues
                # halves its transfer but lengthens the finalize barrier by
                # ~1 us -- a second queue to complete/drain -- so keep one.)
                # The last group's DMA gates the finalize barrier by ~2.9 us
                # (path latency + 16B packets); GpSimd's swdge path and a
                # col-15-only final DMA (4B strided packets) were both
                # tried and are 1.7-2.5 us slower.
                nc.scalar.dma_start(
                    out_d[g, :, :], outsb[:, g * GRP : (g + 1) * GRP]
                )

    nc.compile()
    _dedup_ldweights(nc)
    return nc


def make_in_maps(x, W, b):
    """Host-side shard + preprocess: transpose, fold 0.5 and 32, cast fp8."""
    x = np.asarray(x, dtype=np.float32)
    W = np.asarray(W, dtype=np.float32)
    b = np.asarray(b, dtype=np.float32)
    f8 = ml_dtypes.float8_e4m3

    xt = np.ascontiguousarray(x.T).astype(f8)  # [I, B]
    wt = np.ascontiguousarray(W.T * (np.float32(SCALE) * WS)).astype(f8)  # [I, O]
    bias = (b * (np.float32(SCALE) * WS)).reshape(1, O)

    # Per-batch-half x slabs: [KP, NG, P, GRP, 2, P]; contraction index
    # (kp*2 + i)*128 + p matches the wt slabs below.
    x_slabs = []
    for g in range(NB_G):
        xg = xt[:, g * BC : (g + 1) * BC]  # [I, BC]
        xr = np.ascontiguousarray(
            xg.reshape(KP, 2, P, NG, GRP, P).transpose(0, 3, 2, 4, 1, 5)
        )
        x_slabs.append(xr)
    # Per-out-feature-quarter W slabs [KP, P, NO_MM, 2, NMM] and replicated
    # bias rows.
    w_slabs = []
    b_slabs = []
    ta_slabs = []
    for j in range(NO_G):
        w_slabs.append(
            np.ascontiguousarray(
                wt[:, j * OC : (j + 1) * OC]
                .reshape(KP, 2, P, NO_MM, NMM)
                .transpose(0, 2, 3, 1, 4)
            )
        )
        b_slabs.append(
            np.ascontiguousarray(
                np.broadcast_to(bias[:, j * OC : (j + 1) * OC], (P, OC))
            )
        )
        ta = np.zeros((P, 2, P + 2 * NMM), dtype=np.float32)
        ta[0, 0, 0:P] = 1.0
        ta[0, 0, P:] = bias[0, j * OC + OC // 2 : (j + 1) * OC]
        ta_slabs.append(ta.astype(f8))

    in_maps = []
    for c in range(NB_G * NO_G):
        g, j = divmod(c, NO_G)
        in_maps.append(
            {
                "xt": x_slabs[g],
                "wt": w_slabs[j],
                "biasrep": b_slabs[j],
                "tailaux": ta_slabs[j],
            }
        )
    return in_maps


def combine_outputs(results):
    """Sum the 4 out-feature partials per batch half -> full [B] output."""
    out = np.zeros(B, dtype=np.float32)
    for c, r in enumerate(results):
        g = c // NO_G
        part = np.asarray(r["out"], dtype=np.float32)  # [NG, P, GRP]
        # batch index within the core = (grp*GRP + bb)*P + p
        out[g * BC : (g + 1) * BC] += part.transpose(0, 2, 1).reshape(BC)
    return out / WS  # undo the fp8 weight scale


def kernel(x, W, b):
    from concourse.bass_utils import run_bass_kernel_spmd

    if "nc" not in _NC_CACHE:
        _NC_CACHE["nc"] = build_bass()
    nc = _NC_CACHE["nc"]
    in_maps = make_in_maps(x, W, b)
    res = run_bass_kernel_spmd(nc, in_maps, core_ids=list(range(NB_G * NO_G)))
    return combine_outputs(res.results)
